# revision 8
# baseline (speedup 1.0000x reference)
"""Trainium2 Bass kernel for nn_Attention_1 (B=32, T=2048, H=1024, D_OUT=128).

Math: score = (hs @ W_score) @ h_t is reassociated as hs @ (W_score @ h_t),
turning the dominant [B*T,H]@[H,H] matmul into a per-sample matvec. The
kernel is then HBM-bound: each core streams its 4 samples' hidden_states
(32 MB) through SBUF exactly once.

Sharding: data-parallel over batch, 4 samples per core across 8 cores.
"""

import numpy as np
from contextlib import ExitStack

import concourse.bass as bass
import concourse.bacc as bacc
import concourse.mybir as mybir
from concourse import tile
from concourse import bass_utils
from concourse.masks import make_identity
from concourse import bass_isa

F32 = mybir.dt.float32
B, T, H, DOUT = 32, 2048, 1024, 128
NCORES = 8
BL = B // NCORES     # 4 samples per core
P = 128
NT = T // P          # 16 t-tiles per sample
NH = H // P          # 8 h-chunks
NPA = 2 * H // P     # 16 k-chunks of pre-activation
HS_BUFS = 28         # SBUF slots of [128,1024] f32 for hidden_state tiles


def _emit(ctx: ExitStack, tc: "tile.TileContext", hs_d, ws_d, wo_d, out_d):
    nc = tc.nc

    const = ctx.enter_context(tc.tile_pool(name="const", bufs=1))
    wtp = ctx.enter_context(tc.tile_pool(name="wtp", bufs=1))
    wnp = ctx.enter_context(tc.tile_pool(name="wnp", bufs=2))
    hsp = ctx.enter_context(tc.tile_pool(name="hsp", bufs=HS_BUFS))
    wrk = ctx.enter_context(tc.tile_pool(name="wrk", bufs=2))
    sml = ctx.enter_context(tc.tile_pool(name="sml", bufs=2))
    ps_mm = ctx.enter_context(tc.tile_pool(name="ps_mm", bufs=4, space="PSUM"))
    ps_ctx = ctx.enter_context(tc.tile_pool(name="ps_ctx", bufs=2, space="PSUM"))

    identity = const.tile([P, P], F32, tag="ident")
    make_identity(nc, identity[:])

    # W_out [2048,128] -> wo[k_part, c, n]
    wo = const.tile([P, NPA, DOUT], F32, tag="wo")
    for c in range(NPA):
        nc.sync.dma_start(wo[:, c, :], wo_d[c * P:(c + 1) * P, :])

    # h_t columns: ht[p, c, b] = hs[b, T-1, c*128+p]
    ht = const.tile([P, NH, BL], F32, tag="ht")
    for b in range(BL):
        nc.sync.dma_start(
            ht[:, :, b], hs_d[b, T - 1, :].rearrange("(c p) -> p c", p=P)
        )

    # Transpose W_score via PE: wts[kc][k_part, h_free] = W_score[h, kc*128+k]
    wts = [
        wtp.tile([P, H], F32, tag=f"wt{kc}", name=f"wt{kc}") for kc in range(NH)
    ]
    for hc in range(NH):
        wn = wnp.tile([P, H], F32, tag="wn")
        nc.sync.dma_start(wn[:], ws_d[hc * P:(hc + 1) * P, :])
        for kc in range(NH):
            tps = ps_mm.tile([P, P], F32, tag="mm")
            nc.tensor.transpose(tps[:], wn[:, kc * P:(kc + 1) * P], identity[:])
            nc.vector.tensor_copy(wts[kc][:, hc * P:(hc + 1) * P], tps[:])

    # v[b, :] = W_score @ h_t[b]  -> [BL, H] (rows)
    v_sb = const.tile([BL, H], F32, tag="vsb")
    for n2 in range(2):
        pv = ps_mm.tile([BL, 512], F32, tag="mm")
        for kc in range(NH):
            nc.tensor.matmul(
                pv[:],
                ht[:, kc, :],
                wts[kc][:, n2 * 512:(n2 + 1) * 512],
                start=(kc == 0),
                stop=(kc == NH - 1),
            )
        nc.vector.tensor_copy(v_sb[:, n2 * 512:(n2 + 1) * 512], pv[:])

    # pre-activation lhsT: pa[k_part, c, b]
    pa = const.tile([P, NPA, BL], F32, tag="pa")

    for b in range(BL):
        # move v[b] to a partition-0 row, then broadcast across partitions
        vrow = wrk.tile([1, H], F32, tag="vrow")
        nc.sync.dma_start(vrow[:], v_sb[b:b + 1, :])
        vb = wrk.tile([P, H], F32, tag="vb")
        nc.gpsimd.partition_broadcast(vb[:], vrow[:])

        hst = []
        for i in range(NT):
            t_ = hsp.tile([P, H], F32, tag="hst")
            nc.sync.dma_start(t_[:], hs_d[b, i * P:(i + 1) * P, :])
            hst.append(t_)

        # score[t] = hs[b, t, :] . v[b]   (fused mul+reduce on DVE)
        score = sml.tile([P, NT], F32, tag="score")
        for i in range(NT):
            prod = wrk.tile([P, H], F32, tag="prod")
            nc.vector.tensor_tensor(
                out=prod[:], in0=hst[i][:], in1=vb[:], op=mybir.AluOpType.mult
            )
            nc.scalar.activation(
                prod[:], prod[:], mybir.ActivationFunctionType.Copy,
                accum_out=score[:, i:i + 1],
            )

        # numerically-stable softmax over all 2048 scores
        m1 = sml.tile([P, 1], F32, tag="m1")
        nc.vector.tensor_reduce(
            m1[:], score[:], axis=mybir.AxisListType.X, op=mybir.AluOpType.max
        )
        gma = sml.tile([P, 1], F32, tag="gma")
        nc.gpsimd.partition_all_reduce(
            gma[:], m1[:], channels=P, reduce_op=bass_isa.ReduceOp.max
        )
        gmn = sml.tile([P, 1], F32, tag="gmn")
        nc.scalar.mul(gmn[:], gma[:], -1.0)

        e = sml.tile([P, NT], F32, tag="e")
        ssum = sml.tile([P, 1], F32, tag="ssum")
        nc.scalar.activation(
            e[:], score[:], mybir.ActivationFunctionType.Exp,
            bias=gmn[:], scale=1.0, accum_out=ssum[:],
        )
        sa = sml.tile([P, 1], F32, tag="sa")
        nc.gpsimd.partition_all_reduce(
            sa[:], ssum[:], channels=P, reduce_op=bass_isa.ReduceOp.add
        )
        reca = sml.tile([P, 1], F32, tag="reca")
        nc.vector.reciprocal(reca[:], sa[:])

        w = sml.tile([P, NT], F32, tag="w")
        nc.vector.tensor_scalar_mul(w[:], e[:], reca[:])

        # context^T columns: ctxp[k, hc] = sum_t w[t] * hs[b, t, hc*128+k]
        ctxp = ps_ctx.tile([P, NH], F32, tag="ctx")
        for hc in range(NH):
            for i in range(NT):
                nc.tensor.matmul(
                    ctxp[:, hc:hc + 1],
                    hst[i][:, hc * P:(hc + 1) * P],
                    w[:, i:i + 1],
                    start=(i == 0),
                    stop=(i == NT - 1),
                )
        nc.vector.tensor_copy(pa[:, 0:NH, b], ctxp[:])
        nc.vector.tensor_copy(pa[:, NH:NPA, b], ht[:, :, b])

    # attention_vector = tanh(pre_act @ W_out), batched over the 4 samples
    ops_ = ps_mm.tile([BL, DOUT], F32, tag="mm")
    for c in range(NPA):
        nc.tensor.matmul(
            ops_[:], pa[:, c, :], wo[:, c, :],
            start=(c == 0), stop=(c == NPA - 1),
        )
    res = sml.tile([BL, DOUT], F32, tag="res")
    nc.scalar.activation(res[:], ops_[:], mybir.ActivationFunctionType.Tanh)
    nc.sync.dma_start(out_d[:, :], res[:])


_CACHE = None


def build():
    global _CACHE
    if _CACHE is None:
        nc = bacc.Bacc(
            "TRN2", target_bir_lowering=False, debug=False, num_devices=NCORES
        )
        hs_d = nc.dram_tensor("hs", [BL, T, H], F32, kind="ExternalInput").ap()
        ws_d = nc.dram_tensor("w_score", [H, H], F32, kind="ExternalInput").ap()
        wo_d = nc.dram_tensor("w_out", [2 * H, DOUT], F32, kind="ExternalInput").ap()
        out_d = nc.dram_tensor("out", [BL, DOUT], F32, kind="ExternalOutput").ap()
        with tile.TileContext(nc) as tc:
            with ExitStack() as ctx:
                _emit(ctx, tc, hs_d, ws_d, wo_d, out_d)
        nc.compile()
        _CACHE = nc
    return _CACHE


def make_in_maps(hidden_states, W_score, W_out):
    hs = np.ascontiguousarray(np.asarray(hidden_states, dtype=np.float32))
    ws = np.ascontiguousarray(np.asarray(W_score, dtype=np.float32))
    wo = np.ascontiguousarray(np.asarray(W_out, dtype=np.float32))
    return [
        {"hs": hs[c * BL:(c + 1) * BL], "w_score": ws, "w_out": wo}
        for c in range(NCORES)
    ]


def kernel(hidden_states, W_score, W_out):
    nc = build()
    in_maps = make_in_maps(hidden_states, W_score, W_out)
    res = bass_utils.run_bass_kernel_spmd(nc, in_maps, core_ids=list(range(NCORES)))
    return np.concatenate([r["out"] for r in res.results], axis=0)


if __name__ == "__main__":
    import jax

    with jax.default_device(jax.devices("cpu")[0]):
        key = jax.random.key(0)
        k1, k2, k3 = jax.random.split(key, 3)
        hs = np.asarray(jax.random.normal(k1, (B, T, H), dtype=np.float32))
    out = kernel(hs, np.eye(H, dtype=np.float32), np.ones((2 * H, DOUT), np.float32))
    print(out.shape, out.dtype)


# revision 13
# speedup vs baseline: 1.6733x; 1.6733x over previous
"""Trainium2 Bass kernel for nn_Attention_1 (B=32, T=2048, H=1024, D_OUT=128).

Math: score = (hs @ W_score) @ h_t is reassociated as hs @ (W_score @ h_t),
turning the dominant [B*T,H]@[H,H] matmul into a per-sample matvec. The
kernel is then HBM-bound: each core streams its 4 samples' hidden_states
(32 MB) through SBUF exactly once.

Sharding: data-parallel over batch, 4 samples per core across 8 cores.
"""

import numpy as np
from contextlib import ExitStack

import concourse.bass as bass
import concourse.bacc as bacc
import concourse.mybir as mybir
from concourse import tile
from concourse import bass_utils
from concourse.masks import make_identity
from concourse import bass_isa

F32 = mybir.dt.float32
B, T, H, DOUT = 32, 2048, 1024, 128
NCORES = 8
BL = B // NCORES     # 4 samples per core
P = 128
NT = T // P          # 16 t-tiles per sample
NH = H // P          # 8 h-chunks
NPA = 2 * H // P     # 16 k-chunks of pre-activation
HS_BUFS = 28         # SBUF slots of [128,1024] f32 for hidden_state tiles
USE_F32R = False      # float32r (single-pass PE fp32) for the context matmul


def _emit(ctx: ExitStack, tc: "tile.TileContext", hs_d, ws_d, wo_d, out_d, scr_d):
    nc = tc.nc

    const = ctx.enter_context(tc.tile_pool(name="const", bufs=1))
    wtp = ctx.enter_context(tc.tile_pool(name="wtp", bufs=1))
    wnp = ctx.enter_context(tc.tile_pool(name="wnp", bufs=2))
    hsp = ctx.enter_context(tc.tile_pool(name="hsp", bufs=HS_BUFS))
    wrk = ctx.enter_context(tc.tile_pool(name="wrk", bufs=2))
    sml = ctx.enter_context(tc.tile_pool(name="sml", bufs=2))
    ps_mm = ctx.enter_context(tc.tile_pool(name="ps_mm", bufs=4, space="PSUM"))
    ps_cr = ctx.enter_context(tc.tile_pool(name="ps_cr", bufs=4, space="PSUM"))

    identity = const.tile([P, P], F32, tag="ident")
    make_identity(nc, identity[:])

    # W_out [2048,128] -> wo[k_part, c, n]
    wo = const.tile([P, NPA, DOUT], F32, tag="wo")
    for c in range(NPA):
        nc.sync.dma_start(wo[:, c, :], wo_d[c * P:(c + 1) * P, :])

    # h_t columns: ht[p, c, b] = hs[b, T-1, c*128+p]
    ht = const.tile([P, NH, BL], F32, tag="ht")
    for b in range(BL):
        nc.sync.dma_start(
            ht[:, :, b], hs_d[b, T - 1, :].rearrange("(c p) -> p c", p=P)
        )

    # Transpose W_score via PE: wts[kc][k_part, h_free] = W_score[h, kc*128+k]
    wts = [
        wtp.tile([P, H], F32, tag=f"wt{kc}", name=f"wt{kc}") for kc in range(NH)
    ]
    for hc in range(NH):
        wn = wnp.tile([P, H], F32, tag="wn")
        nc.sync.dma_start(wn[:], ws_d[hc * P:(hc + 1) * P, :])
        for kc in range(NH):
            tps = ps_mm.tile([P, P], F32, tag="mm")
            nc.tensor.transpose(tps[:], wn[:, kc * P:(kc + 1) * P], identity[:])
            nc.vector.tensor_copy(wts[kc][:, hc * P:(hc + 1) * P], tps[:])

    # v[b, :] = W_score @ h_t[b]  -> [BL, H] (rows)
    v_sb = const.tile([BL, H], F32, tag="vsb")
    for n2 in range(2):
        pv = ps_mm.tile([BL, 512], F32, tag="mm")
        for kc in range(NH):
            nc.tensor.matmul(
                pv[:],
                ht[:, kc, :],
                wts[kc][:, n2 * 512:(n2 + 1) * 512],
                start=(kc == 0),
                stop=(kc == NH - 1),
            )
        nc.vector.tensor_copy(v_sb[:, n2 * 512:(n2 + 1) * 512], pv[:])

    # pre-activation lhsT: pa[k_part, c, b]
    pa = const.tile([P, NPA, BL], F32, tag="pa")

    for b in range(BL):
        # move v[b] to a partition-0 row, then broadcast across partitions
        vrow = wrk.tile([1, H], F32, tag="vrow")
        nc.sync.dma_start(vrow[:], v_sb[b:b + 1, :])
        vb = wrk.tile([P, H], F32, tag="vb")
        nc.gpsimd.partition_broadcast(vb[:], vrow[:])

        hst = []
        for i in range(NT):
            t_ = hsp.tile([P, H], F32, tag="hst")
            nc.sync.dma_start(t_[:], hs_d[b, i * P:(i + 1) * P, :])
            hst.append(t_)

        # score[t] = hs[b, t, :] . v[b]   (fused mul+reduce on DVE)
        score = sml.tile([P, NT], F32, tag="score")
        for i in range(NT):
            prod = wrk.tile([P, H], F32, tag="prod")
            nc.vector.tensor_tensor(
                out=prod[:], in0=hst[i][:], in1=vb[:], op=mybir.AluOpType.mult
            )
            nc.scalar.activation(
                prod[:], prod[:], mybir.ActivationFunctionType.Copy,
                accum_out=score[:, i:i + 1],
            )

        # numerically-stable softmax over all 2048 scores
        m1 = sml.tile([P, 1], F32, tag="m1")
        nc.vector.tensor_reduce(
            m1[:], score[:], axis=mybir.AxisListType.X, op=mybir.AluOpType.max
        )
        gma = sml.tile([P, 1], F32, tag="gma")
        nc.gpsimd.partition_all_reduce(
            gma[:], m1[:], channels=P, reduce_op=bass_isa.ReduceOp.max
        )
        gmn = sml.tile([P, 1], F32, tag="gmn")
        nc.scalar.mul(gmn[:], gma[:], -1.0)

        e = sml.tile([P, NT], F32, tag="e")
        ssum = sml.tile([P, 1], F32, tag="ssum")
        nc.scalar.activation(
            e[:], score[:], mybir.ActivationFunctionType.Exp,
            bias=gmn[:], scale=1.0, accum_out=ssum[:],
        )
        sa = sml.tile([P, 1], F32, tag="sa")
        nc.gpsimd.partition_all_reduce(
            sa[:], ssum[:], channels=P, reduce_op=bass_isa.ReduceOp.add
        )
        reca = sml.tile([P, 1], F32, tag="reca")
        nc.vector.reciprocal(reca[:], sa[:])

        w = sml.tile([P, NT], F32, tag="w")
        nc.vector.tensor_scalar_mul(w[:], e[:], reca[:])

        # context row: ctx[h] = sum_t w[t] * hs[b, t, h]
        # stationary = w column (tiny load), moving = hs tile at N=512
        mm_dt = mybir.dt.float32r if USE_F32R else F32
        cr0 = ps_cr.tile([1, 512], F32, tag="cr")
        cr1 = ps_cr.tile([1, 512], F32, tag="cr")
        for i in range(NT):
            lw = w[:, i:i + 1].bitcast(mm_dt)
            rh = hst[i][:].bitcast(mm_dt)
            nc.tensor.matmul(
                cr0[:], lw, rh[:, 0:512], start=(i == 0), stop=(i == NT - 1)
            )
            nc.tensor.matmul(
                cr1[:], lw, rh[:, 512:H], start=(i == 0), stop=(i == NT - 1)
            )
        ctxrow = sml.tile([1, H], F32, tag="ctxrow")
        nc.vector.tensor_copy(ctxrow[:, 0:512], cr0[:])
        nc.vector.tensor_copy(ctxrow[:, 512:H], cr1[:])
        # scatter the row into pa's column layout via a DRAM bounce
        nc.sync.dma_start(scr_d[b:b + 1, :], ctxrow[0:1, :])
        nc.sync.dma_start(
            pa[:, 0:NH, b], scr_d[b, :].rearrange("(c p) -> p c", p=P)
        )
        nc.vector.tensor_copy(pa[:, NH:NPA, b], ht[:, :, b])

    # attention_vector = tanh(pre_act @ W_out), batched over the 4 samples
    ops_ = ps_mm.tile([BL, DOUT], F32, tag="mm")
    for c in range(NPA):
        nc.tensor.matmul(
            ops_[:], pa[:, c, :], wo[:, c, :],
            start=(c == 0), stop=(c == NPA - 1),
        )
    res = sml.tile([BL, DOUT], F32, tag="res")
    nc.scalar.activation(res[:], ops_[:], mybir.ActivationFunctionType.Tanh)
    nc.sync.dma_start(out_d[:, :], res[:])


_CACHE = None


def build():
    global _CACHE
    if _CACHE is None:
        nc = bacc.Bacc(
            "TRN2", target_bir_lowering=False, debug=False, num_devices=NCORES
        )
        hs_d = nc.dram_tensor("hs", [BL, T, H], F32, kind="ExternalInput").ap()
        ws_d = nc.dram_tensor("w_score", [H, H], F32, kind="ExternalInput").ap()
        wo_d = nc.dram_tensor("w_out", [2 * H, DOUT], F32, kind="ExternalInput").ap()
        out_d = nc.dram_tensor("out", [BL, DOUT], F32, kind="ExternalOutput").ap()
        scr_d = nc.dram_tensor(
            "scratch", [BL, H], F32, kind="ExternalOutput"
        ).ap()
        with tile.TileContext(nc) as tc:
            with ExitStack() as ctx:
                _emit(ctx, tc, hs_d, ws_d, wo_d, out_d, scr_d)
        nc.compile()
        _CACHE = nc
    return _CACHE


def make_in_maps(hidden_states, W_score, W_out):
    hs = np.ascontiguousarray(np.asarray(hidden_states, dtype=np.float32))
    ws = np.ascontiguousarray(np.asarray(W_score, dtype=np.float32))
    wo = np.ascontiguousarray(np.asarray(W_out, dtype=np.float32))
    return [
        {"hs": hs[c * BL:(c + 1) * BL], "w_score": ws, "w_out": wo}
        for c in range(NCORES)
    ]


def kernel(hidden_states, W_score, W_out):
    nc = build()
    in_maps = make_in_maps(hidden_states, W_score, W_out)
    res = bass_utils.run_bass_kernel_spmd(nc, in_maps, core_ids=list(range(NCORES)))
    return np.concatenate([r["out"] for r in res.results], axis=0)


if __name__ == "__main__":
    import jax

    with jax.default_device(jax.devices("cpu")[0]):
        key = jax.random.key(0)
        k1, k2, k3 = jax.random.split(key, 3)
        hs = np.asarray(jax.random.normal(k1, (B, T, H), dtype=np.float32))
    out = kernel(hs, np.eye(H, dtype=np.float32), np.ones((2 * H, DOUT), np.float32))
    print(out.shape, out.dtype)
